# revision 38
# baseline (speedup 1.0000x reference)
"""BiAttention (BiDAF-style) kernel for Trainium2, 8 NeuronCores.

Reference math (T=4096, d=512):
    context  = x[0,0]; question = x[1,0]
    S[i,j]   = w1.c_i + w2.q_j + (c_i*w3).q_j
    A        = softmax_j(S)          # w1.c_i is constant per row -> cancels
    U_A      = A @ question
    b        = max_j A[i,j]
    h        = b @ context           # global over T -> one AllReduce
    G        = [context, U_A, context*U_A, context*h]

Sharding: context rows (rows of S/A/U_A/G) split across 8 cores (512 rows
each); question replicated; h all-reduced (1 KB fp16).

Critical-path architecture (v2): the end-to-end critical chain is
  head DMA -> S^T phase (PE-bound, ~21us) -> row-max fold -> b ->
  h partial -> DRAM -> AllReduce -> broadcast load -> c*h -> store,
where the h round-trip costs ~6.5us of pure DMA/semaphore latency.
Everything else (U_A matmuls, 1/Z scaling, c*U_A products, G stores)
is scheduled UNDER that latency umbrella:

  - Phase 1 is S-only (3-term compensated fp8 DoubleRow product, same
    numerics as v1) at 1-jt granularity ([128,512] PSUM tiles, 3 bufs)
    so only 3 PSUM banks are needed and Z (ones-column DoubleRow
    matmuls, ~1 cycle each) accumulates in its own bank DURING phase 1
    (lagged 2 pairs behind the slow Pool casts) -> zinv is ready the
    moment the last cast lands.
  - The running row-max is a single fp16 [128,512] DVE chain (2x mode)
    updated per jt; the tail is just 4 fp16 transposes (PE) + a PSUM
    reduce + b16.
  - Phase 2 PE stream: UA(0) | T Z14 Z15 reduce zinv b16 | UA(1) |
    h matmuls | UA(2..15): the b->h chain launches ~2.5us after the
    last S matmul while 6.8us of U_A matmuls and all G-block stores
    fill the AllReduce round-trip.  The final running-max update is
    split in half so the transposes pipeline behind it.
  - U_A scale-copies split ACT/DVE (2+2) so the per-ic
    copy->product->store pipeline drains ~1us faster.
  - All G stores ride the sync queue EMITTED AFTER the h broadcast
    load, so they can never steal HWDGE/DMA slots from the
    latency-critical h path; G block 0 (context) goes straight from
    the c16 input tile mid-phase-1.
  - Head: cw8 | packed qtt[j 0:512] (2 KB/partition rows, no sub-512B
    DMA penalty) | cwr8, so the first S matmul issues at ~5us.
"""

import numpy as np
import ml_dtypes

import concourse.bass as bass
import concourse.mybir as mybir
import concourse.tile as tile
from concourse import bacc
from concourse.bass_utils import run_bass_kernel_spmd
from concourse.masks import make_identity

F32 = mybir.dt.float32
F16 = mybir.dt.float16
F8 = mybir.dt.float8e4
AF = mybir.ActivationFunctionType
DR = mybir.MatmulPerfMode.DoubleRow
NP8 = ml_dtypes.float8_e4m3

T = 4096
D = 512
NCORES = 8
TL = T // NCORES          # 512 local context rows per core
P = 128
NIC = TL // P             # 4 i-chunks of 128
NJT = T // P              # 32 j-tiles of 128
NPAIR = NJT // 2          # 16 j-tile pairs (DoubleRow contraction unit)
SHIFT = 3.0               # global exp shift; cancels in softmax/b


def build_kernel(collective=True, compile=True):
    nc = bacc.Bacc("TRN2", target_bir_lowering=False, debug=False,
                   num_devices=NCORES if collective else 1)

    qtt_d = nc.dram_tensor("qtt", [P, 8, T], F8, kind="ExternalInput").ap()
    qhd_d = nc.dram_tensor("qhd", [P, 8, 512], F8, kind="ExternalInput").ap()
    qn8_d = nc.dram_tensor("qn8", [P, NJT, D], F8, kind="ExternalInput").ap()
    cwp_d = nc.dram_tensor("cwp", [P, 8, TL], F8, kind="ExternalInput").ap()
    c16_d = nc.dram_tensor("c16", [P, NIC, D], F16, kind="ExternalInput").ap()
    g_d = nc.dram_tensor("g", [TL, 4 * D], F16, kind="ExternalOutput").ap()

    with tile.TileContext(nc) as tc:
        _emit(nc, tc, qtt_d, qhd_d, qn8_d, cwp_d, c16_d, g_d,
              collective=collective)

    if compile:
        nc.compile()
    return nc


def _emit(nc, tc, qtt_d, qhd_d, qn8_d, cwp_d, c16_d, g_d,
          collective=True):
    from contextlib import ExitStack
    ctx = ExitStack()
    consts = ctx.enter_context(tc.tile_pool(name="consts", bufs=1))
    gpool = ctx.enter_context(tc.tile_pool(name="gpool", bufs=1))
    spool = ctx.enter_context(tc.tile_pool(name="spool", bufs=3, space="PSUM"))
    uapool = ctx.enter_context(tc.tile_pool(name="uapool", bufs=4, space="PSUM"))
    zpool = ctx.enter_context(tc.tile_pool(name="zpool", bufs=1, space="PSUM"))
    dram = ctx.enter_context(tc.tile_pool(name="dram", bufs=1, space="DRAM"))

    # ---- head DMAs first: the sync SEQ must issue these ASAP --------------
    cwp = consts.tile([P, 8, TL], F8)
    qhd = consts.tile([P, 8, 512], F8)
    nc.sync.dma_start(out=cwp[:, 0:4], in_=cwp_d[:, 0:4])
    nc.sync.dma_start(out=qhd, in_=qhd_d)
    nc.sync.dma_start(out=cwp[:, 4:8], in_=cwp_d[:, 4:8])
    cw8 = cwp[:, 0:4]
    cwr8 = cwp[:, 4:8]

    # ---- prologue: PE p-state anchor + constants --------------------------
    # Warm-up matmuls: anchor the PE p-state ramp at t~0 so the real
    # S matmuls (first data at ~5us) run at full clock from the start.
    wa = consts.tile([P, P], F16)
    nc.vector.memset(wa, 0.0)
    wb = consts.tile([P, 256], F16)
    nc.vector.memset(wb, 0.0)
    zh = zpool.tile([P, D], F32, tag="zh", name="zh")
    for _ in range(8):
        nc.tensor.matmul(zh[:, 256:512], lhsT=wa, rhs=wb,
                         start=True, stop=True, skip_group_check=True)

    bias_t = consts.tile([P, 1], F32)
    nc.vector.memset(bias_t, -SHIFT)
    ones8 = consts.tile([P, 2, 1], F8)
    nc.vector.memset(ones8, 1.0)
    ones16 = consts.tile([P, 1], F16)
    nc.vector.memset(ones16, 1.0)
    ident16 = consts.tile([P, P], F16)
    make_identity(nc, ident16)
    # dummy exp warms the ACT table (free in the cost model, real on HW)
    warm = consts.tile([1, 1], F32)
    nc.vector.memset(warm, 0.0)
    nc.scalar.activation(out=warm, in_=warm, func=AF.Exp)

    # ---- inputs -----------------------------------------------------------
    # Remaining input DMAs, sync (SP) queue, in need-order: qtt slices
    # (the S stream), then qn8-s0 (phase-2 U_A), c16 (h partial + G
    # blocks), rest of qn8.
    qtt = consts.tile([P, 8, T], F8)
    # 512-wide j slices: inner contiguous run is 512 B, the smallest size
    # that avoids the sub-512B DMA read-modify-write penalty.  Slice 0 is
    # covered by the packed head tensor.
    for s in range(1, 8):
        js = slice(s * 512, (s + 1) * 512)
        nc.sync.dma_start(out=qtt[:, :, js], in_=qtt_d[:, :, js])
    qn8 = consts.tile([P, NJT, D], F8)
    c16 = consts.tile([P, NIC, D], F16)
    nc.sync.dma_start(out=qn8[:, 0:8], in_=qn8_d[:, 0:8])
    nc.sync.dma_start(out=c16, in_=c16_d)
    for s in range(1, 4):
        jc = slice(8 * s, 8 * s + 8)
        nc.sync.dma_start(out=qn8[:, jc], in_=qn8_d[:, jc])
    # G block 0 (context) goes straight from the input tile, mid-phase-1
    # while the DMA engines are otherwise idle.
    nc.scalar.dma_start(
        out=g_d.rearrange("(ic p) c -> p ic c", p=P)[:, :, 0:D],
        in_=c16)

    # ---- persistent phase-1 state ----------------------------------------
    e16 = consts.tile([P, NJT, D], F16)
    e8 = consts.tile([P, NJT, D], F8)
    # single fp16 running-max accumulator, updated per jt (DVE 2x mode);
    # E > 0 so 0-init is safe in the E domain.
    macc = consts.tile([P, D], F16)
    nc.vector.memset(macc, 0.0)

    nz = [0]

    def emit_z(p):
        # Z(pair p) = sum_j E^T[j, i] via ones-column DoubleRow matmuls:
        # out free size 1 -> ~1 cycle each.  Accumulates in zh[:, 0:4].
        for ic in range(NIC):
            nc.tensor.matmul(zh[:, ic:ic + 1],
                             lhsT=e8[:, 2 * p:2 * p + 2, ic * P:(ic + 1) * P],
                             rhs=ones8,
                             start=(nz[0] == 0), stop=False,
                             perf_mode=DR, skip_group_check=True)
            nz[0] += 1

    # ---- phase 1: S^T -> exp -> running max / casts / Z -------------------
    # Per jt: 6 fp8 DoubleRow matmuls (3-term compensated product, K=256
    # per instruction) into a [128,512] PSUM tile; exp on ACT (the only
    # PSUM reader); fp16 running max on DVE; fp16->fp8 cast on DVE/Pool
    # alternating (last two jt both DVE so the tail never waits on Pool).
    for jt in range(NJT):
        hp = tc.high_priority()
        hp.__enter__()
        qsrc = qhd if jt < 4 else qtt
        qs8 = qsrc[:, 0:4]
        qsr8 = qsrc[:, 4:8]
        jj = slice((jt % 4 if jt < 4 else jt) * P,
                   ((jt % 4 if jt < 4 else jt) + 1) * P)
        st = spool.tile([P, D], F32, tag="s", name=f"st{jt}")
        first = True
        for (lhs, rhs) in ((qs8, cw8), (qsr8, cw8), (qs8, cwr8)):
            for a in range(2):
                nc.tensor.matmul(
                    st,
                    lhsT=lhs[:, 2 * a:2 * a + 2, jj],
                    rhs=rhs[:, 2 * a:2 * a + 2, :],
                    start=first, stop=(rhs is cwr8 and a == 1),
                    perf_mode=DR)
                first = False
        nc.scalar.activation(out=e16[:, jt], in_=st, func=AF.Exp, bias=bias_t)
        nc.vector.tensor_tensor(out=macc, in0=e16[:, jt], in1=macc,
                                op=mybir.AluOpType.max)
        # strict DVE/Pool alternation: jt31's cast rides Pool so the DVE
        # queue stays clear for the latency-critical max/reduce/b16 chain
        eng = nc.vector if jt % 2 == 0 else nc.gpsimd
        eng.tensor_copy(out=e8[:, jt], in_=e16[:, jt])
        # Z lags two pairs so its (Pool-latency) casts are long done
        if jt % 2 == 1 and jt >= 5:
            emit_z((jt - 1) // 2 - 2)
        hp.__exit__(None, None, None)

    # ---- phase 2 + tail ---------------------------------------------------
    # PE stream order: UA(0) Z14 UA(1) T Z15 UA(2) UA(3) h UA(4..15).
    # The b->h->AllReduce->c*h chain is emitted high-priority so its DMAs
    # and vector ops never park behind the (slack-rich) G work.
    ua_ps = [uapool.tile([P, D], F32, tag="ua", name=f"ua{ic}")
             for ic in range(NIC)]

    def emit_ua(p):
        for ic in range(NIC):
            nc.tensor.matmul(ua_ps[ic],
                             lhsT=e8[:, 2 * p:2 * p + 2, ic * P:(ic + 1) * P],
                             rhs=qn8[:, 2 * p:2 * p + 2, :],
                             start=(p == 0), stop=(p == NPAIR - 1),
                             perf_mode=DR, skip_group_check=True)

    emit_ua(0)

    with tc.high_priority():
        # per-i row max: transpose the fp16 max accumulator (i to
        # partitions) then reduce over the old partition axis
        tp = spool.tile([P, D], F16, tag="s", name="tp_m")
        for ic in range(NIC):
            nc.tensor.transpose(tp[:, ic * P:(ic + 1) * P],
                                macc[:, ic * P:(ic + 1) * P], ident16)
        emit_z(NPAIR - 2)
        # Z for the last pair reads e16 directly (fp16, K=128): it waits
        # only on exp31, not on the slower Pool cast, so zinv lands
        # before the reduce finishes
        for jt in (NJT - 2, NJT - 1):
            for ic in range(NIC):
                nc.tensor.matmul(zh[:, ic:ic + 1],
                                 lhsT=e16[:, jt, ic * P:(ic + 1) * P],
                                 rhs=ones16,
                                 start=False,
                                 stop=(jt == NJT - 1 and ic == NIC - 1),
                                 skip_group_check=True)
        emax = consts.tile([P, NIC], F32)
        nc.vector.tensor_reduce(out=emax,
                                in_=tp.rearrange("p (ic q) -> p ic q", q=P),
                                axis=mybir.AxisListType.X,
                                op=mybir.AluOpType.max)
        zinv = consts.tile([P, NIC], F32)
        nc.vector.reciprocal(out=zinv, in_=zh[:, 0:NIC])
        b16 = consts.tile([P, NIC], F16)
        nc.vector.tensor_tensor(out=b16, in0=emax, in1=zinv,
                                op=mybir.AluOpType.mult)

    emit_ua(1)

    with tc.high_priority():
        # h partial into zh[:, 8:12]; store -> AllReduce -> broadcast load
        for ic in range(NIC):
            for dc in range(4):
                nc.tensor.matmul(zh[:, 8 + dc:9 + dc],
                                 lhsT=c16[:, ic, dc * P:(dc + 1) * P],
                                 rhs=b16[:, ic:ic + 1],
                                 start=(ic == 0 and dc == 0),
                                 stop=(ic == NIC - 1 and dc == 3),
                                 skip_group_check=True)
        h_sb = consts.tile([P, 4], F16)
        nc.vector.tensor_copy(out=h_sb, in_=zh[:, 8:12])
        hp_dram = dram.tile([D], F16)
        hs_dram = dram.tile([D], F16)
        hp_ap = hp_dram[:]
        nc.sync.dma_start(out=hp_ap.rearrange("(dc p) -> p dc", p=P),
                          in_=h_sb)
        if collective:
            nc.gpsimd.collective_compute(
                "AllReduce", mybir.AluOpType.add,
                replica_groups=[list(range(NCORES))],
                ins=[hp_dram.opt()], outs=[hs_dram.opt()],
            )
        else:
            nc.sync.dma_start(out=hs_dram[:], in_=hp_dram[:])
        hs_ap = hs_dram[:]
        h_bc = consts.tile([P, D], F16)
        nc.sync.dma_start(
            out=h_bc,
            in_=bass.AP(tensor=hs_ap.tensor, offset=hs_ap.offset,
                        ap=[[0, P], [1, D]]),
        )

    for p in range(2, NPAIR):
        emit_ua(p)

    # ---- G blocks 1-2 (U_A, c*U_A): per-ic scale-copy, product, store ----
    # gst[p, ic, :]: 0:512 = U_A, 512:1024 = c*U_A, 1024:1536 = c*h.
    # Scale-copies split ACT (ic 0,2) / DVE (ic 1,3); products on DVE;
    # stores on the sync queue (emitted after the h load above, so the
    # h path always wins HWDGE/DMA arbitration).
    gst = gpool.tile([P, NIC, 3 * D], F16)
    for ic in range(NIC):
        if ic % 2 == 0:
            nc.scalar.activation(out=gst[:, ic, 0:D], in_=ua_ps[ic],
                                 func=AF.Copy, scale=zinv[:, ic:ic + 1])
        else:
            nc.vector.tensor_scalar(out=gst[:, ic, 0:D], in0=ua_ps[ic],
                                    scalar1=zinv[:, ic:ic + 1], scalar2=None,
                                    op0=mybir.AluOpType.mult)
        nc.vector.tensor_tensor(out=gst[:, ic, D:2 * D],
                                in0=c16[:, ic, :], in1=gst[:, ic, 0:D],
                                op=mybir.AluOpType.mult)
        nc.sync.dma_start(out=g_d[ic * P:(ic + 1) * P, D:3 * D],
                          in_=gst[:, ic, 0:2 * D])

    # ---- G block 3 (c*h): 2 paired products + 2 paired stores ------------
    h_bc2 = bass.AP(tensor=h_bc.tensor, offset=h_bc.offset,
                    ap=[h_bc.ap[0], [0, 2], h_bc.ap[1]])
    g3d = g_d.rearrange("(ic p) c -> p ic c", p=P)
    with tc.high_priority():
        for icp in range(2):
            nc.vector.tensor_tensor(
                out=gst[:, 2 * icp:2 * icp + 2, 2 * D:3 * D],
                in0=c16[:, 2 * icp:2 * icp + 2, :], in1=h_bc2,
                op=mybir.AluOpType.mult)
            nc.sync.dma_start(
                out=g3d[:, 2 * icp:2 * icp + 2, 3 * D:4 * D],
                in_=gst[:, 2 * icp:2 * icp + 2, 2 * D:3 * D])

    ctx.close()


# ---------------------------------------------------------------------------


def _prep_inputs(x, w):
    """Host-side quantization + layout. Returns per-core in_maps."""
    context = np.ascontiguousarray(x[0, 0]).astype(np.float32)   # (T, D)
    question = np.ascontiguousarray(x[1, 0]).astype(np.float32)  # (T, D)
    w = np.asarray(w, dtype=np.float32)
    w2 = w[D:2 * D]
    w3 = w[2 * D:3 * D]

    # question.T in [p, dc, j] layout, fp8 + fp8 residual
    qT = question.T.reshape(4, P, T)                  # [dc, p, j]
    qT = np.ascontiguousarray(qT.transpose(1, 0, 2))  # [p, dc, j]
    qt8 = qT.astype(NP8)
    qtr8 = (qT - qt8.astype(np.float32)).astype(NP8)
    qtt = np.concatenate([qt8, qtr8], axis=1)         # [p, 8, j]
    qhd = np.ascontiguousarray(qtt[:, :, 0:512])      # packed head slice

    # question natural in [p, jc, d] layout, fp8
    qn = question.reshape(NJT, P, D)                  # [jc, p, d]
    qn8 = np.ascontiguousarray(qn.transpose(1, 0, 2)).astype(NP8)

    cw_full = context * w3[None, :] + w2[None, :]     # (T, D)

    in_maps = []
    for core in range(NCORES):
        rows = slice(core * TL, (core + 1) * TL)
        cw = cw_full[rows]                            # (TL, D)
        cwT = cw.T.reshape(4, P, TL)                  # [dc, p, i]
        cwT = np.ascontiguousarray(cwT.transpose(1, 0, 2))
        cw8 = cwT.astype(NP8)
        cwr8 = (cwT - cw8.astype(np.float32)).astype(NP8)
        cwp = np.concatenate([cw8, cwr8], axis=1)     # [p, 8, i]
        cn = context[rows].reshape(NIC, P, D)         # [ic, p, d]
        c16 = np.ascontiguousarray(cn.transpose(1, 0, 2)).astype(np.float16)
        in_maps.append({
            "qtt": qtt, "qhd": qhd, "qn8": qn8, "cwp": cwp, "c16": c16,
        })
    return in_maps


_NC_CACHE = {}


def _get_nc():
    if "nc" not in _NC_CACHE:
        _NC_CACHE["nc"] = build_kernel()
    return _NC_CACHE["nc"]


def kernel(x: np.ndarray, kernel: np.ndarray) -> np.ndarray:
    nc = _get_nc()
    in_maps = _prep_inputs(x, kernel)
    res = run_bass_kernel_spmd(nc, in_maps, core_ids=list(range(NCORES)))
    g = np.concatenate([res.results[core]["g"] for core in range(NCORES)],
                       axis=0)
    return g.astype(np.float32)


# revision 40
# speedup vs baseline: 1.0496x; 1.0496x over previous
"""BiAttention (BiDAF-style) kernel for Trainium2, 8 NeuronCores.

Reference math (T=4096, d=512):
    context  = x[0,0]; question = x[1,0]
    S[i,j]   = w1.c_i + w2.q_j + (c_i*w3).q_j
    A        = softmax_j(S)          # w1.c_i is constant per row -> cancels
    U_A      = A @ question
    b        = max_j A[i,j]
    h        = b @ context           # global over T -> one AllReduce
    G        = [context, U_A, context*U_A, context*h]

Sharding: context rows (rows of S/A/U_A/G) split across 8 cores (512 rows
each); question replicated; h all-reduced (1 KB fp16).

Critical-path architecture (v2): the end-to-end critical chain is
  head DMA -> S^T phase (PE-bound, ~21us) -> row-max fold -> b ->
  h partial -> DRAM -> AllReduce -> broadcast load -> c*h -> store,
where the h round-trip costs ~6.5us of pure DMA/semaphore latency.
Everything else (U_A matmuls, 1/Z scaling, c*U_A products, G stores)
is scheduled UNDER that latency umbrella:

  - Phase 1 is S-only (3-term compensated fp8 DoubleRow product, same
    numerics as v1) at 1-jt granularity ([128,512] PSUM tiles, 3 bufs)
    so only 3 PSUM banks are needed and Z (ones-column DoubleRow
    matmuls, ~1 cycle each) accumulates in its own bank DURING phase 1
    (lagged 2 pairs behind the slow Pool casts) -> zinv is ready the
    moment the last cast lands.
  - The running row-max is a single fp16 [128,512] DVE chain (2x mode)
    updated per jt; the tail is just 4 fp16 transposes (PE) + a PSUM
    reduce + b16.
  - Phase 2 PE stream: UA(0) | T Z14 Z15 reduce zinv b16 | UA(1) |
    h matmuls | UA(2..15): the b->h chain launches ~2.5us after the
    last S matmul while 6.8us of U_A matmuls and all G-block stores
    fill the AllReduce round-trip.  The final running-max update is
    split in half so the transposes pipeline behind it.
  - U_A scale-copies split ACT/DVE (2+2) so the per-ic
    copy->product->store pipeline drains ~1us faster.
  - All G stores ride the sync queue EMITTED AFTER the h broadcast
    load, so they can never steal HWDGE/DMA slots from the
    latency-critical h path; G block 0 (context) goes straight from
    the c16 input tile mid-phase-1.
  - Head: cw8 | packed qtt[j 0:512] (2 KB/partition rows, no sub-512B
    DMA penalty) | cwr8, so the first S matmul issues at ~5us.
"""

import numpy as np
import ml_dtypes

import concourse.bass as bass
import concourse.mybir as mybir
import concourse.tile as tile
from concourse import bacc
from concourse.bass_utils import run_bass_kernel_spmd
from concourse.masks import make_identity

F32 = mybir.dt.float32
F16 = mybir.dt.float16
F8 = mybir.dt.float8e4
AF = mybir.ActivationFunctionType
DR = mybir.MatmulPerfMode.DoubleRow
NP8 = ml_dtypes.float8_e4m3

T = 4096
D = 512
NCORES = 8
TL = T // NCORES          # 512 local context rows per core
P = 128
NIC = TL // P             # 4 i-chunks of 128
NJT = T // P              # 32 j-tiles of 128
NPAIR = NJT // 2          # 16 j-tile pairs (DoubleRow contraction unit)
SHIFT = 3.0               # global exp shift; cancels in softmax/b


def build_kernel(collective=True, compile=True):
    nc = bacc.Bacc("TRN2", target_bir_lowering=False, debug=False,
                   num_devices=NCORES if collective else 1)

    qtt_d = nc.dram_tensor("qtt", [P, 8, T], F8, kind="ExternalInput").ap()
    qhd_d = nc.dram_tensor("qhd", [P, 8, 512], F8, kind="ExternalInput").ap()
    qn8_d = nc.dram_tensor("qn8", [P, NJT, D], F8, kind="ExternalInput").ap()
    cwp_d = nc.dram_tensor("cwp", [P, 8, TL], F8, kind="ExternalInput").ap()
    c16_d = nc.dram_tensor("c16", [P, NIC, D], F16, kind="ExternalInput").ap()
    g_d = nc.dram_tensor("g", [TL, 4 * D], F16, kind="ExternalOutput").ap()

    with tile.TileContext(nc) as tc:
        _emit(nc, tc, qtt_d, qhd_d, qn8_d, cwp_d, c16_d, g_d,
              collective=collective)

    if compile:
        nc.compile()
    return nc


def _emit(nc, tc, qtt_d, qhd_d, qn8_d, cwp_d, c16_d, g_d,
          collective=True):
    from contextlib import ExitStack
    ctx = ExitStack()
    consts = ctx.enter_context(tc.tile_pool(name="consts", bufs=1))
    gpool = ctx.enter_context(tc.tile_pool(name="gpool", bufs=1))
    spool = ctx.enter_context(tc.tile_pool(name="spool", bufs=3, space="PSUM"))
    uapool = ctx.enter_context(tc.tile_pool(name="uapool", bufs=4, space="PSUM"))
    zpool = ctx.enter_context(tc.tile_pool(name="zpool", bufs=1, space="PSUM"))
    dram = ctx.enter_context(tc.tile_pool(name="dram", bufs=1, space="DRAM"))

    # ---- head DMAs first: the sync SEQ must issue these ASAP --------------
    cwp = consts.tile([P, 8, TL], F8)
    qhd = consts.tile([P, 8, 512], F8)
    nc.sync.dma_start(out=cwp[:, 0:4], in_=cwp_d[:, 0:4])
    nc.sync.dma_start(out=qhd, in_=qhd_d)
    nc.sync.dma_start(out=cwp[:, 4:8], in_=cwp_d[:, 4:8])
    cw8 = cwp[:, 0:4]
    cwr8 = cwp[:, 4:8]

    # ---- prologue: PE p-state anchor + constants --------------------------
    # Warm-up matmuls: anchor the PE p-state ramp at t~0 so the real
    # S matmuls (first data at ~5us) run at full clock from the start.
    wa = consts.tile([P, P], F16)
    nc.vector.memset(wa, 0.0)
    wb = consts.tile([P, 256], F16)
    nc.vector.memset(wb, 0.0)
    zh = zpool.tile([P, D], F32, tag="zh", name="zh")
    for _ in range(8):
        nc.tensor.matmul(zh[:, 256:512], lhsT=wa, rhs=wb,
                         start=True, stop=True, skip_group_check=True)

    bias_t = consts.tile([P, 1], F32)
    nc.vector.memset(bias_t, -SHIFT)
    ones8 = consts.tile([P, 2, 1], F8)
    nc.vector.memset(ones8, 1.0)
    ones16 = consts.tile([P, 1], F16)
    nc.vector.memset(ones16, 1.0)
    ident16 = consts.tile([P, P], F16)
    make_identity(nc, ident16)
    # dummy exp warms the ACT table (free in the cost model, real on HW)
    warm = consts.tile([1, 1], F32)
    nc.vector.memset(warm, 0.0)
    nc.scalar.activation(out=warm, in_=warm, func=AF.Exp)

    # ---- inputs -----------------------------------------------------------
    # Remaining input DMAs, sync (SP) queue, in need-order: qtt slices
    # (the S stream), then qn8-s0 (phase-2 U_A), c16 (h partial + G
    # blocks), rest of qn8.
    qtt = consts.tile([P, 8, T], F8)
    # 512-wide j slices: inner contiguous run is 512 B, the smallest size
    # that avoids the sub-512B DMA read-modify-write penalty.  Slice 0 is
    # covered by the packed head tensor.
    for s in range(1, 8):
        js = slice(s * 512, (s + 1) * 512)
        nc.sync.dma_start(out=qtt[:, :, js], in_=qtt_d[:, :, js])
    qn8 = consts.tile([P, NJT, D], F8)
    c16 = consts.tile([P, NIC, D], F16)
    nc.sync.dma_start(out=qn8[:, 0:8], in_=qn8_d[:, 0:8])
    nc.sync.dma_start(out=c16, in_=c16_d)
    for s in range(1, 4):
        jc = slice(8 * s, 8 * s + 8)
        nc.sync.dma_start(out=qn8[:, jc], in_=qn8_d[:, jc])
    # G block 0 (context) goes straight from the input tile, mid-phase-1
    # while the DMA engines are otherwise idle.
    nc.scalar.dma_start(
        out=g_d.rearrange("(ic p) c -> p ic c", p=P)[:, :, 0:D],
        in_=c16)

    # ---- persistent phase-1 state ----------------------------------------
    e16 = consts.tile([P, NJT, D], F16)
    e8 = consts.tile([P, NJT, D], F8)
    # single fp16 running-max accumulator, updated per jt (DVE 2x mode);
    # E > 0 so 0-init is safe in the E domain.
    macc = consts.tile([P, D], F16)
    nc.vector.memset(macc, 0.0)

    nz = [0]

    def emit_z(p):
        # Z(pair p) = sum_j E^T[j, i] via ones-column DoubleRow matmuls:
        # out free size 1 -> ~1 cycle each.  Accumulates in zh[:, 0:4].
        for ic in range(NIC):
            nc.tensor.matmul(zh[:, ic:ic + 1],
                             lhsT=e8[:, 2 * p:2 * p + 2, ic * P:(ic + 1) * P],
                             rhs=ones8,
                             start=(nz[0] == 0), stop=False,
                             perf_mode=DR, skip_group_check=True)
            nz[0] += 1

    # ---- phase 1: S^T -> exp -> running max / casts / Z -------------------
    # Per jt: 6 fp8 DoubleRow matmuls (3-term compensated product, K=256
    # per instruction) into a [128,512] PSUM tile; exp on ACT (the only
    # PSUM reader); fp16 running max on DVE; fp16->fp8 cast on DVE/Pool
    # alternating (last two jt both DVE so the tail never waits on Pool).
    for jt in range(NJT):
        qsrc = qhd if jt < 4 else qtt
        qs8 = qsrc[:, 0:4]
        qsr8 = qsrc[:, 4:8]
        jj = slice((jt % 4 if jt < 4 else jt) * P,
                   ((jt % 4 if jt < 4 else jt) + 1) * P)
        st = spool.tile([P, D], F32, tag="s", name=f"st{jt}")
        first = True
        for (lhs, rhs) in ((qs8, cw8), (qsr8, cw8), (qs8, cwr8)):
            for a in range(2):
                nc.tensor.matmul(
                    st,
                    lhsT=lhs[:, 2 * a:2 * a + 2, jj],
                    rhs=rhs[:, 2 * a:2 * a + 2, :],
                    start=first, stop=(rhs is cwr8 and a == 1),
                    perf_mode=DR)
                first = False
        nc.scalar.activation(out=e16[:, jt], in_=st, func=AF.Exp, bias=bias_t)
        nc.vector.tensor_tensor(out=macc, in0=e16[:, jt], in1=macc,
                                op=mybir.AluOpType.max)
        # strict DVE/Pool alternation: jt31's cast rides Pool so the DVE
        # queue stays clear for the latency-critical max/reduce/b16 chain
        eng = nc.vector if jt % 2 == 0 else nc.gpsimd
        eng.tensor_copy(out=e8[:, jt], in_=e16[:, jt])
        # Z lags two pairs so its (Pool-latency) casts are long done
        if jt % 2 == 1 and jt >= 5:
            emit_z((jt - 1) // 2 - 2)

    # ---- phase 2 + tail ---------------------------------------------------
    # PE stream order: UA(0) Z14 UA(1) T Z15 UA(2) UA(3) h UA(4..15).
    # The b->h->AllReduce->c*h chain is emitted high-priority so its DMAs
    # and vector ops never park behind the (slack-rich) G work.
    ua_ps = [uapool.tile([P, D], F32, tag="ua", name=f"ua{ic}")
             for ic in range(NIC)]

    def emit_ua(p):
        # demoted priority: the scheduler must never hoist U_A matmuls
        # ahead of the b-chain-gating S matmuls
        with tc.high_priority(offset=-1000000):
            for ic in range(NIC):
                nc.tensor.matmul(ua_ps[ic],
                                 lhsT=e8[:, 2 * p:2 * p + 2,
                                         ic * P:(ic + 1) * P],
                                 rhs=qn8[:, 2 * p:2 * p + 2, :],
                                 start=(p == 0), stop=(p == NPAIR - 1),
                                 perf_mode=DR, skip_group_check=True)

    emit_ua(0)

    with tc.high_priority():
        # per-i row max: transpose the fp16 max accumulator (i to
        # partitions) then reduce over the old partition axis
        tp = spool.tile([P, D], F16, tag="s", name="tp_m")
        for ic in range(NIC):
            nc.tensor.transpose(tp[:, ic * P:(ic + 1) * P],
                                macc[:, ic * P:(ic + 1) * P], ident16)
        emit_z(NPAIR - 2)
        # Z for the last pair reads e16 directly (fp16, K=128): it waits
        # only on exp31, not on the slower Pool cast, so zinv lands
        # before the reduce finishes
        for jt in (NJT - 2, NJT - 1):
            for ic in range(NIC):
                nc.tensor.matmul(zh[:, ic:ic + 1],
                                 lhsT=e16[:, jt, ic * P:(ic + 1) * P],
                                 rhs=ones16,
                                 start=False,
                                 stop=(jt == NJT - 1 and ic == NIC - 1),
                                 skip_group_check=True)
        emax = consts.tile([P, NIC], F32)
        nc.vector.tensor_reduce(out=emax,
                                in_=tp.rearrange("p (ic q) -> p ic q", q=P),
                                axis=mybir.AxisListType.X,
                                op=mybir.AluOpType.max)
        zinv = consts.tile([P, NIC], F32)
        nc.vector.reciprocal(out=zinv, in_=zh[:, 0:NIC])
        b16 = consts.tile([P, NIC], F16)
        nc.vector.tensor_tensor(out=b16, in0=emax, in1=zinv,
                                op=mybir.AluOpType.mult)

    emit_ua(1)

    with tc.high_priority():
        # h partial into zh[:, 8:12]; store -> AllReduce -> broadcast load
        for ic in range(NIC):
            for dc in range(4):
                nc.tensor.matmul(zh[:, 8 + dc:9 + dc],
                                 lhsT=c16[:, ic, dc * P:(dc + 1) * P],
                                 rhs=b16[:, ic:ic + 1],
                                 start=(ic == 0 and dc == 0),
                                 stop=(ic == NIC - 1 and dc == 3),
                                 skip_group_check=True)
        h_sb = consts.tile([P, 4], F16)
        nc.vector.tensor_copy(out=h_sb, in_=zh[:, 8:12])
        hp_dram = dram.tile([D], F16)
        hs_dram = dram.tile([D], F16)
        hp_ap = hp_dram[:]
        nc.sync.dma_start(out=hp_ap.rearrange("(dc p) -> p dc", p=P),
                          in_=h_sb)
        if collective:
            nc.gpsimd.collective_compute(
                "AllReduce", mybir.AluOpType.add,
                replica_groups=[list(range(NCORES))],
                ins=[hp_dram.opt()], outs=[hs_dram.opt()],
            )
        else:
            nc.sync.dma_start(out=hs_dram[:], in_=hp_dram[:])
        hs_ap = hs_dram[:]
        h_bc = consts.tile([P, D], F16)
        nc.sync.dma_start(
            out=h_bc,
            in_=bass.AP(tensor=hs_ap.tensor, offset=hs_ap.offset,
                        ap=[[0, P], [1, D]]),
        )

    for p in range(2, NPAIR):
        emit_ua(p)

    # ---- G blocks 1-2 (U_A, c*U_A): per-ic scale-copy, product, store ----
    # gst[p, ic, :]: 0:512 = U_A, 512:1024 = c*U_A, 1024:1536 = c*h.
    # Scale-copies split ACT (ic 0,2) / DVE (ic 1,3); products on DVE;
    # stores on the sync queue (emitted after the h load above, so the
    # h path always wins HWDGE/DMA arbitration).
    gst = gpool.tile([P, NIC, 3 * D], F16)
    for ic in range(NIC):
        if ic % 2 == 0:
            nc.scalar.activation(out=gst[:, ic, 0:D], in_=ua_ps[ic],
                                 func=AF.Copy, scale=zinv[:, ic:ic + 1])
        else:
            nc.vector.tensor_scalar(out=gst[:, ic, 0:D], in0=ua_ps[ic],
                                    scalar1=zinv[:, ic:ic + 1], scalar2=None,
                                    op0=mybir.AluOpType.mult)
        nc.vector.tensor_tensor(out=gst[:, ic, D:2 * D],
                                in0=c16[:, ic, :], in1=gst[:, ic, 0:D],
                                op=mybir.AluOpType.mult)
        nc.sync.dma_start(out=g_d[ic * P:(ic + 1) * P, D:3 * D],
                          in_=gst[:, ic, 0:2 * D])

    # ---- G block 3 (c*h): 2 paired products + 2 paired stores ------------
    h_bc2 = bass.AP(tensor=h_bc.tensor, offset=h_bc.offset,
                    ap=[h_bc.ap[0], [0, 2], h_bc.ap[1]])
    g3d = g_d.rearrange("(ic p) c -> p ic c", p=P)
    with tc.high_priority():
        for icp in range(2):
            nc.vector.tensor_tensor(
                out=gst[:, 2 * icp:2 * icp + 2, 2 * D:3 * D],
                in0=c16[:, 2 * icp:2 * icp + 2, :], in1=h_bc2,
                op=mybir.AluOpType.mult)
            nc.sync.dma_start(
                out=g3d[:, 2 * icp:2 * icp + 2, 3 * D:4 * D],
                in_=gst[:, 2 * icp:2 * icp + 2, 2 * D:3 * D])

    ctx.close()


# ---------------------------------------------------------------------------


def _prep_inputs(x, w):
    """Host-side quantization + layout. Returns per-core in_maps."""
    context = np.ascontiguousarray(x[0, 0]).astype(np.float32)   # (T, D)
    question = np.ascontiguousarray(x[1, 0]).astype(np.float32)  # (T, D)
    w = np.asarray(w, dtype=np.float32)
    w2 = w[D:2 * D]
    w3 = w[2 * D:3 * D]

    # question.T in [p, dc, j] layout, fp8 + fp8 residual
    qT = question.T.reshape(4, P, T)                  # [dc, p, j]
    qT = np.ascontiguousarray(qT.transpose(1, 0, 2))  # [p, dc, j]
    qt8 = qT.astype(NP8)
    qtr8 = (qT - qt8.astype(np.float32)).astype(NP8)
    qtt = np.concatenate([qt8, qtr8], axis=1)         # [p, 8, j]
    qhd = np.ascontiguousarray(qtt[:, :, 0:512])      # packed head slice

    # question natural in [p, jc, d] layout, fp8
    qn = question.reshape(NJT, P, D)                  # [jc, p, d]
    qn8 = np.ascontiguousarray(qn.transpose(1, 0, 2)).astype(NP8)

    cw_full = context * w3[None, :] + w2[None, :]     # (T, D)

    in_maps = []
    for core in range(NCORES):
        rows = slice(core * TL, (core + 1) * TL)
        cw = cw_full[rows]                            # (TL, D)
        cwT = cw.T.reshape(4, P, TL)                  # [dc, p, i]
        cwT = np.ascontiguousarray(cwT.transpose(1, 0, 2))
        cw8 = cwT.astype(NP8)
        cwr8 = (cwT - cw8.astype(np.float32)).astype(NP8)
        cwp = np.concatenate([cw8, cwr8], axis=1)     # [p, 8, i]
        cn = context[rows].reshape(NIC, P, D)         # [ic, p, d]
        c16 = np.ascontiguousarray(cn.transpose(1, 0, 2)).astype(np.float16)
        in_maps.append({
            "qtt": qtt, "qhd": qhd, "qn8": qn8, "cwp": cwp, "c16": c16,
        })
    return in_maps


_NC_CACHE = {}


def _get_nc():
    if "nc" not in _NC_CACHE:
        _NC_CACHE["nc"] = build_kernel()
    return _NC_CACHE["nc"]


def kernel(x: np.ndarray, kernel: np.ndarray) -> np.ndarray:
    nc = _get_nc()
    in_maps = _prep_inputs(x, kernel)
    res = run_bass_kernel_spmd(nc, in_maps, core_ids=list(range(NCORES)))
    g = np.concatenate([res.results[core]["g"] for core in range(NCORES)],
                       axis=0)
    return g.astype(np.float32)


# revision 41
# speedup vs baseline: 1.0958x; 1.0440x over previous
"""BiAttention (BiDAF-style) kernel for Trainium2, 8 NeuronCores.

Reference math (T=4096, d=512):
    context  = x[0,0]; question = x[1,0]
    S[i,j]   = w1.c_i + w2.q_j + (c_i*w3).q_j
    A        = softmax_j(S)          # w1.c_i is constant per row -> cancels
    U_A      = A @ question
    b        = max_j A[i,j]
    h        = b @ context           # global over T -> one AllReduce
    G        = [context, U_A, context*U_A, context*h]

Sharding: context rows (rows of S/A/U_A/G) split across 8 cores (512 rows
each); question replicated; h all-reduced (1 KB fp16).

Critical-path architecture (v2): the end-to-end critical chain is
  head DMA -> S^T phase (PE-bound, ~21us) -> row-max fold -> b ->
  h partial -> DRAM -> AllReduce -> broadcast load -> c*h -> store,
where the h round-trip costs ~6.5us of pure DMA/semaphore latency.
Everything else (U_A matmuls, 1/Z scaling, c*U_A products, G stores)
is scheduled UNDER that latency umbrella:

  - Phase 1 is S-only (3-term compensated fp8 DoubleRow product, same
    numerics as v1) at 1-jt granularity ([128,512] PSUM tiles, 3 bufs)
    so only 3 PSUM banks are needed and Z (ones-column DoubleRow
    matmuls, ~1 cycle each) accumulates in its own bank DURING phase 1
    (lagged 2 pairs behind the slow Pool casts) -> zinv is ready the
    moment the last cast lands.
  - The running row-max is a single fp16 [128,512] DVE chain (2x mode)
    updated per jt; the tail is just 4 fp16 transposes (PE) + a PSUM
    reduce + b16.
  - Phase 2 PE stream: UA(0) | T Z14 Z15 reduce zinv b16 | UA(1) |
    h matmuls | UA(2..15): the b->h chain launches ~2.5us after the
    last S matmul while 6.8us of U_A matmuls and all G-block stores
    fill the AllReduce round-trip.  The final running-max update is
    split in half so the transposes pipeline behind it.
  - U_A scale-copies split ACT/DVE (2+2) so the per-ic
    copy->product->store pipeline drains ~1us faster.
  - All G stores ride the sync queue EMITTED AFTER the h broadcast
    load, so they can never steal HWDGE/DMA slots from the
    latency-critical h path; G block 0 (context) goes straight from
    the c16 input tile mid-phase-1.
  - Head: cw8 | packed qtt[j 0:512] (2 KB/partition rows, no sub-512B
    DMA penalty) | cwr8, so the first S matmul issues at ~5us.
"""

import numpy as np
import ml_dtypes

import concourse.bass as bass
import concourse.mybir as mybir
import concourse.tile as tile
from concourse import bacc
from concourse.bass_utils import run_bass_kernel_spmd
from concourse.masks import make_identity

F32 = mybir.dt.float32
F16 = mybir.dt.float16
F8 = mybir.dt.float8e4
AF = mybir.ActivationFunctionType
DR = mybir.MatmulPerfMode.DoubleRow
NP8 = ml_dtypes.float8_e4m3

T = 4096
D = 512
NCORES = 8
TL = T // NCORES          # 512 local context rows per core
P = 128
NIC = TL // P             # 4 i-chunks of 128
NJT = T // P              # 32 j-tiles of 128
NPAIR = NJT // 2          # 16 j-tile pairs (DoubleRow contraction unit)
SHIFT = 3.0               # global exp shift; cancels in softmax/b


def build_kernel(collective=True, compile=True):
    nc = bacc.Bacc("TRN2", target_bir_lowering=False, debug=False,
                   num_devices=NCORES if collective else 1)

    qtt_d = nc.dram_tensor("qtt", [P, 8, T], F8, kind="ExternalInput").ap()
    qhd_d = nc.dram_tensor("qhd", [P, 8, 512], F8, kind="ExternalInput").ap()
    qn8_d = nc.dram_tensor("qn8", [P, NJT, D], F8, kind="ExternalInput").ap()
    cwp_d = nc.dram_tensor("cwp", [P, 8, TL], F8, kind="ExternalInput").ap()
    c16_d = nc.dram_tensor("c16", [P, NIC, D], F16, kind="ExternalInput").ap()
    g_d = nc.dram_tensor("g", [TL, 4 * D], F16, kind="ExternalOutput").ap()

    with tile.TileContext(nc) as tc:
        _emit(nc, tc, qtt_d, qhd_d, qn8_d, cwp_d, c16_d, g_d,
              collective=collective)

    if compile:
        nc.compile()
    return nc


def _emit(nc, tc, qtt_d, qhd_d, qn8_d, cwp_d, c16_d, g_d,
          collective=True):
    from contextlib import ExitStack
    ctx = ExitStack()
    consts = ctx.enter_context(tc.tile_pool(name="consts", bufs=1))
    gpool = ctx.enter_context(tc.tile_pool(name="gpool", bufs=1))
    spool = ctx.enter_context(tc.tile_pool(name="spool", bufs=3, space="PSUM"))
    uapool = ctx.enter_context(tc.tile_pool(name="uapool", bufs=4, space="PSUM"))
    zpool = ctx.enter_context(tc.tile_pool(name="zpool", bufs=1, space="PSUM"))
    dram = ctx.enter_context(tc.tile_pool(name="dram", bufs=1, space="DRAM"))

    # ---- head DMAs first: the sync SEQ must issue these ASAP --------------
    cwp = consts.tile([P, 8, TL], F8)
    qhd = consts.tile([P, 8, 512], F8)
    nc.sync.dma_start(out=cwp[:, 0:4], in_=cwp_d[:, 0:4])
    nc.sync.dma_start(out=qhd, in_=qhd_d)
    nc.sync.dma_start(out=cwp[:, 4:8], in_=cwp_d[:, 4:8])
    cw8 = cwp[:, 0:4]
    cwr8 = cwp[:, 4:8]

    # ---- prologue: PE p-state anchor + constants --------------------------
    # Warm-up matmuls: anchor the PE p-state ramp at t~0 so the real
    # S matmuls (first data at ~5us) run at full clock from the start.
    wa = consts.tile([P, P], F16)
    nc.vector.memset(wa, 0.0)
    wb = consts.tile([P, 256], F16)
    nc.vector.memset(wb, 0.0)
    zh = zpool.tile([P, D], F32, tag="zh", name="zh")
    for _ in range(8):
        nc.tensor.matmul(zh[:, 256:512], lhsT=wa, rhs=wb,
                         start=True, stop=True, skip_group_check=True)

    bias_t = consts.tile([P, 1], F32)
    nc.vector.memset(bias_t, -SHIFT)
    ones8 = consts.tile([P, 2, 1], F8)
    nc.vector.memset(ones8, 1.0)
    ones16 = consts.tile([P, 1], F16)
    nc.vector.memset(ones16, 1.0)
    ident16 = consts.tile([P, P], F16)
    make_identity(nc, ident16)
    # dummy exp warms the ACT table (free in the cost model, real on HW)
    warm = consts.tile([1, 1], F32)
    nc.vector.memset(warm, 0.0)
    nc.scalar.activation(out=warm, in_=warm, func=AF.Exp)

    # ---- inputs -----------------------------------------------------------
    # Remaining input DMAs, sync (SP) queue, in need-order: qtt slices
    # (the S stream), then qn8-s0 (phase-2 U_A), c16 (h partial + G
    # blocks), rest of qn8.
    qtt = consts.tile([P, 8, T], F8)
    # 512-wide j slices: inner contiguous run is 512 B, the smallest size
    # that avoids the sub-512B DMA read-modify-write penalty.  Slice 0 is
    # covered by the packed head tensor.
    for s in range(1, 8):
        js = slice(s * 512, (s + 1) * 512)
        nc.sync.dma_start(out=qtt[:, :, js], in_=qtt_d[:, :, js])
    qn8 = consts.tile([P, NJT, D], F8)
    c16 = consts.tile([P, NIC, D], F16)
    nc.sync.dma_start(out=c16, in_=c16_d)
    # qn8 slice 0 LAST: U_A(0..3) only become data-ready near the end of
    # phase 1, so the list scheduler cannot hoist U_A matmuls into S-phase
    # stall slots (that hoisting delays S31 and with it the whole b->h
    # critical chain)
    for s in (1, 2, 3, 0):
        jc = slice(8 * s, 8 * s + 8)
        nc.sync.dma_start(out=qn8[:, jc], in_=qn8_d[:, jc])
    # G block 0 (context) goes straight from the input tile, mid-phase-1
    # while the DMA engines are otherwise idle.
    nc.scalar.dma_start(
        out=g_d.rearrange("(ic p) c -> p ic c", p=P)[:, :, 0:D],
        in_=c16)

    # ---- persistent phase-1 state ----------------------------------------
    e16 = consts.tile([P, NJT, D], F16)
    e8 = consts.tile([P, NJT, D], F8)
    # single fp16 running-max accumulator, updated per jt (DVE 2x mode);
    # E > 0 so 0-init is safe in the E domain.
    macc = consts.tile([P, D], F16)
    nc.vector.memset(macc, 0.0)

    nz = [0]

    def emit_z(p):
        # Z(pair p) = sum_j E^T[j, i] via ones-column DoubleRow matmuls:
        # out free size 1 -> ~1 cycle each.  Accumulates in zh[:, 0:4].
        for ic in range(NIC):
            nc.tensor.matmul(zh[:, ic:ic + 1],
                             lhsT=e8[:, 2 * p:2 * p + 2, ic * P:(ic + 1) * P],
                             rhs=ones8,
                             start=(nz[0] == 0), stop=False,
                             perf_mode=DR, skip_group_check=True)
            nz[0] += 1

    # ---- phase 1: S^T -> exp -> running max / casts / Z -------------------
    # Per jt: 6 fp8 DoubleRow matmuls (3-term compensated product, K=256
    # per instruction) into a [128,512] PSUM tile; exp on ACT (the only
    # PSUM reader); fp16 running max on DVE; fp16->fp8 cast on DVE/Pool
    # alternating (last two jt both DVE so the tail never waits on Pool).
    for jt in range(NJT):
        qsrc = qhd if jt < 4 else qtt
        qs8 = qsrc[:, 0:4]
        qsr8 = qsrc[:, 4:8]
        jj = slice((jt % 4 if jt < 4 else jt) * P,
                   ((jt % 4 if jt < 4 else jt) + 1) * P)
        st = spool.tile([P, D], F32, tag="s", name=f"st{jt}")
        first = True
        for (lhs, rhs) in ((qs8, cw8), (qsr8, cw8), (qs8, cwr8)):
            for a in range(2):
                nc.tensor.matmul(
                    st,
                    lhsT=lhs[:, 2 * a:2 * a + 2, jj],
                    rhs=rhs[:, 2 * a:2 * a + 2, :],
                    start=first, stop=(rhs is cwr8 and a == 1),
                    perf_mode=DR)
                first = False
        nc.scalar.activation(out=e16[:, jt], in_=st, func=AF.Exp, bias=bias_t)
        nc.vector.tensor_tensor(out=macc, in0=e16[:, jt], in1=macc,
                                op=mybir.AluOpType.max)
        # strict DVE/Pool alternation: jt31's cast rides Pool so the DVE
        # queue stays clear for the latency-critical max/reduce/b16 chain
        eng = nc.vector if jt % 2 == 0 else nc.gpsimd
        eng.tensor_copy(out=e8[:, jt], in_=e16[:, jt])
        # Z lags two pairs so its (Pool-latency) casts are long done
        if jt % 2 == 1 and jt >= 5:
            emit_z((jt - 1) // 2 - 2)

    # ---- phase 2 + tail ---------------------------------------------------
    # PE stream order: UA(0) Z14 UA(1) T Z15 UA(2) UA(3) h UA(4..15).
    # The b->h->AllReduce->c*h chain is emitted high-priority so its DMAs
    # and vector ops never park behind the (slack-rich) G work.
    ua_ps = [uapool.tile([P, D], F32, tag="ua", name=f"ua{ic}")
             for ic in range(NIC)]

    def emit_ua(p):
        for ic in range(NIC):
            nc.tensor.matmul(ua_ps[ic],
                             lhsT=e8[:, 2 * p:2 * p + 2, ic * P:(ic + 1) * P],
                             rhs=qn8[:, 2 * p:2 * p + 2, :],
                             start=(p == 0), stop=(p == NPAIR - 1),
                             perf_mode=DR, skip_group_check=True)

    emit_ua(0)

    with tc.high_priority():
        # per-i row max: transpose the fp16 max accumulator (i to
        # partitions) then reduce over the old partition axis
        tp = spool.tile([P, D], F16, tag="s", name="tp_m")
        for ic in range(NIC):
            nc.tensor.transpose(tp[:, ic * P:(ic + 1) * P],
                                macc[:, ic * P:(ic + 1) * P], ident16)
        emit_z(NPAIR - 2)
        # Z for the last pair reads e16 directly (fp16, K=128): it waits
        # only on exp31, not on the slower Pool cast, so zinv lands
        # before the reduce finishes
        for jt in (NJT - 2, NJT - 1):
            for ic in range(NIC):
                nc.tensor.matmul(zh[:, ic:ic + 1],
                                 lhsT=e16[:, jt, ic * P:(ic + 1) * P],
                                 rhs=ones16,
                                 start=False,
                                 stop=(jt == NJT - 1 and ic == NIC - 1),
                                 skip_group_check=True)
        emax = consts.tile([P, NIC], F32)
        nc.vector.tensor_reduce(out=emax,
                                in_=tp.rearrange("p (ic q) -> p ic q", q=P),
                                axis=mybir.AxisListType.X,
                                op=mybir.AluOpType.max)
        zinv = consts.tile([P, NIC], F32)
        nc.vector.reciprocal(out=zinv, in_=zh[:, 0:NIC])
        b16 = consts.tile([P, NIC], F16)
        nc.vector.tensor_tensor(out=b16, in0=emax, in1=zinv,
                                op=mybir.AluOpType.mult)

    emit_ua(1)

    with tc.high_priority():
        # h partial into zh[:, 8:12]; store -> AllReduce -> broadcast load
        for ic in range(NIC):
            for dc in range(4):
                nc.tensor.matmul(zh[:, 8 + dc:9 + dc],
                                 lhsT=c16[:, ic, dc * P:(dc + 1) * P],
                                 rhs=b16[:, ic:ic + 1],
                                 start=(ic == 0 and dc == 0),
                                 stop=(ic == NIC - 1 and dc == 3),
                                 skip_group_check=True)
        h_sb = consts.tile([P, 4], F16)
        nc.vector.tensor_copy(out=h_sb, in_=zh[:, 8:12])
        hp_dram = dram.tile([D], F16)
        hs_dram = dram.tile([D], F16)
        hp_ap = hp_dram[:]
        nc.sync.dma_start(out=hp_ap.rearrange("(dc p) -> p dc", p=P),
                          in_=h_sb)
        if collective:
            nc.gpsimd.collective_compute(
                "AllReduce", mybir.AluOpType.add,
                replica_groups=[list(range(NCORES))],
                ins=[hp_dram.opt()], outs=[hs_dram.opt()],
            )
        else:
            nc.sync.dma_start(out=hs_dram[:], in_=hp_dram[:])
        hs_ap = hs_dram[:]
        h_bc = consts.tile([P, D], F16)
        nc.sync.dma_start(
            out=h_bc,
            in_=bass.AP(tensor=hs_ap.tensor, offset=hs_ap.offset,
                        ap=[[0, P], [1, D]]),
        )

    for p in range(2, NPAIR):
        emit_ua(p)

    # ---- G blocks 1-2 (U_A, c*U_A): per-ic scale-copy, product, store ----
    # gst[p, ic, :]: 0:512 = U_A, 512:1024 = c*U_A, 1024:1536 = c*h.
    # Scale-copies split ACT (ic 0,2) / DVE (ic 1,3); products on DVE;
    # stores on the sync queue (emitted after the h load above, so the
    # h path always wins HWDGE/DMA arbitration).
    gst = gpool.tile([P, NIC, 3 * D], F16)
    for ic in range(NIC):
        if ic % 2 == 0:
            nc.scalar.activation(out=gst[:, ic, 0:D], in_=ua_ps[ic],
                                 func=AF.Copy, scale=zinv[:, ic:ic + 1])
        else:
            nc.vector.tensor_scalar(out=gst[:, ic, 0:D], in0=ua_ps[ic],
                                    scalar1=zinv[:, ic:ic + 1], scalar2=None,
                                    op0=mybir.AluOpType.mult)
        nc.vector.tensor_tensor(out=gst[:, ic, D:2 * D],
                                in0=c16[:, ic, :], in1=gst[:, ic, 0:D],
                                op=mybir.AluOpType.mult)
        nc.sync.dma_start(out=g_d[ic * P:(ic + 1) * P, D:3 * D],
                          in_=gst[:, ic, 0:2 * D])

    # ---- G block 3 (c*h): 2 paired products + 2 paired stores ------------
    h_bc2 = bass.AP(tensor=h_bc.tensor, offset=h_bc.offset,
                    ap=[h_bc.ap[0], [0, 2], h_bc.ap[1]])
    g3d = g_d.rearrange("(ic p) c -> p ic c", p=P)
    with tc.high_priority():
        for icp in range(2):
            nc.vector.tensor_tensor(
                out=gst[:, 2 * icp:2 * icp + 2, 2 * D:3 * D],
                in0=c16[:, 2 * icp:2 * icp + 2, :], in1=h_bc2,
                op=mybir.AluOpType.mult)
            nc.sync.dma_start(
                out=g3d[:, 2 * icp:2 * icp + 2, 3 * D:4 * D],
                in_=gst[:, 2 * icp:2 * icp + 2, 2 * D:3 * D])

    ctx.close()


# ---------------------------------------------------------------------------


def _prep_inputs(x, w):
    """Host-side quantization + layout. Returns per-core in_maps."""
    context = np.ascontiguousarray(x[0, 0]).astype(np.float32)   # (T, D)
    question = np.ascontiguousarray(x[1, 0]).astype(np.float32)  # (T, D)
    w = np.asarray(w, dtype=np.float32)
    w2 = w[D:2 * D]
    w3 = w[2 * D:3 * D]

    # question.T in [p, dc, j] layout, fp8 + fp8 residual
    qT = question.T.reshape(4, P, T)                  # [dc, p, j]
    qT = np.ascontiguousarray(qT.transpose(1, 0, 2))  # [p, dc, j]
    qt8 = qT.astype(NP8)
    qtr8 = (qT - qt8.astype(np.float32)).astype(NP8)
    qtt = np.concatenate([qt8, qtr8], axis=1)         # [p, 8, j]
    qhd = np.ascontiguousarray(qtt[:, :, 0:512])      # packed head slice

    # question natural in [p, jc, d] layout, fp8
    qn = question.reshape(NJT, P, D)                  # [jc, p, d]
    qn8 = np.ascontiguousarray(qn.transpose(1, 0, 2)).astype(NP8)

    cw_full = context * w3[None, :] + w2[None, :]     # (T, D)

    in_maps = []
    for core in range(NCORES):
        rows = slice(core * TL, (core + 1) * TL)
        cw = cw_full[rows]                            # (TL, D)
        cwT = cw.T.reshape(4, P, TL)                  # [dc, p, i]
        cwT = np.ascontiguousarray(cwT.transpose(1, 0, 2))
        cw8 = cwT.astype(NP8)
        cwr8 = (cwT - cw8.astype(np.float32)).astype(NP8)
        cwp = np.concatenate([cw8, cwr8], axis=1)     # [p, 8, i]
        cn = context[rows].reshape(NIC, P, D)         # [ic, p, d]
        c16 = np.ascontiguousarray(cn.transpose(1, 0, 2)).astype(np.float16)
        in_maps.append({
            "qtt": qtt, "qhd": qhd, "qn8": qn8, "cwp": cwp, "c16": c16,
        })
    return in_maps


_NC_CACHE = {}


def _get_nc():
    if "nc" not in _NC_CACHE:
        _NC_CACHE["nc"] = build_kernel()
    return _NC_CACHE["nc"]


def kernel(x: np.ndarray, kernel: np.ndarray) -> np.ndarray:
    nc = _get_nc()
    in_maps = _prep_inputs(x, kernel)
    res = run_bass_kernel_spmd(nc, in_maps, core_ids=list(range(NCORES)))
    g = np.concatenate([res.results[core]["g"] for core in range(NCORES)],
                       axis=0)
    return g.astype(np.float32)
